# revision 29
# baseline (speedup 1.0000x reference)
"""Trainium2 Bass kernel for CrossAttentionFusion (B=4096, D=1024, H=16, L=2).

Math notes (exact algebra, no approximation of the reference graph):
  - nn.MultiheadAttention with seq_len==1: softmax over a single key is
    exactly 1.0, so attention(xq, xkv) == (xkv @ Wv.T + bv) @ Wo.T + bo.
    Fuse W = Wo@Wv host-side.
  - Self-attention + residual folds into one matmul: X + X@Wsa.T ==
    X @ (I + Wsa).T; layer 0 additionally folds the input projection.
  - v/t share all layer weights -> both modalities run as two 512-row
    column slices of one [feature, row]-transposed activation set.
  - LN is shift-invariant, so every pre-LN matmul weight (and bias) is
    centered host-side with C = I - (1/D) 11^T while the incoming
    residual is itself centered: the pre-LN sum then has exactly zero
    feature-mean and the mean pass of LayerNorm vanishes.  With the
    default trivial gains this leaves LN = multiply by rsqrt(E[y^2]+eps),
    and steps whose consumer is positively homogeneous into the next LN
    (steps 1, 2, 4) need no LN work at all.

Device strategy: pure data-parallel over batch across 8 cores (512 rows
of each modality per core).  Activations live transposed in SBUF as
bf16; weights are pre-packed bf16 lhsT images streamed once per op with
the two 512-row slices computed back-to-back per weight block
(weight-stationary), which halves HBM weight traffic vs slice-major.
Optionally the FFN blocks run in fp8(e4m3) DoubleRow mode (two
contraction rows per PE pass = 2x matmul throughput); fp8 operands are
extra quantized copies so the bf16 residual trunk keeps full precision.
"""

import numpy as np
import ml_dtypes

import concourse.bass as bass
import concourse.mybir as mybir
import concourse.tile as tile
from concourse import bacc
from concourse.bass_utils import run_bass_kernel_spmd

H = 16
EPS = 1e-5
D = 1024
DFF = 4 * D
L = 2
B = 4096
NCORES = 8
BLOC = B // NCORES          # rows per modality per core (512)
R = 2 * BLOC                # rows per core (v | t) = 1024
P = 128
KD = D // P                 # 8 feature chunks
KF = DFF // P               # 32 dff chunks
NP1 = KD // 2               # fp8 DoubleRow pairs over D
NP2 = KF // 2               # fp8 DoubleRow pairs over DFF
F32 = mybir.dt.float32
F32R = mybir.dt.float32r
BF16 = mybir.dt.bfloat16
FP8 = mybir.dt.float8e4
NPBF16 = ml_dtypes.bfloat16
NPFP8 = ml_dtypes.float8_e4m3
WS = 256.0                  # fp8 weight scale (exact power of two)

FP8_FFN = False             # fp8 DoubleRow FFN toggle (2x PE on FFN, but
                            # quantization noise lands ~4e-2 > the 2e-2 gate)

TRACE = False               # test.py flips this for profiling runs
TRACE_KW = {}

_cache = {}


def _img_lhsT(W):
    """W [dout, din] -> bf16 lhsT tile image [128, nm*nk*128], m-major.

    img[:, (m*nk+k)*128 : +128] == W.T[k*128:(k+1)*128, m*128:(m+1)*128]
    """
    dout, din = W.shape
    nk, nm = din // P, dout // P
    A = np.ascontiguousarray(W.T).reshape(nk, P, nm, P)
    img = A.transpose(1, 2, 0, 3).reshape(P, nm * nk * P)
    return np.ascontiguousarray(img).astype(NPBF16)


def _img_lhsT_dr(W, scale=WS):
    """W [dout, din] -> fp8 DoubleRow image [128, nm*nk*128], m-major,
    k-pair minor: per m-block the layout is [pair j][row i in pair][col].
    """
    dout, din = W.shape
    nk, nm = din // P, dout // P
    npair = nk // 2
    A = np.ascontiguousarray(W.T).reshape(npair, 2, P, nm, P)
    img = A.transpose(2, 3, 0, 1, 4).reshape(P, nm * npair * 2 * P)
    return np.ascontiguousarray(img * scale).astype(NPFP8)


def _bcol(b):
    """bias vector [dout] -> per-partition tile [128, dout/128]."""
    return np.ascontiguousarray(np.asarray(b, np.float64).reshape(-1, P).T
                                ).astype(np.float32)


def _skipvar_flags(flags):
    (b_cv, b_ct, b_sa1, b_ca0, b_ca1, b_f10, b_f11, b_f20, b_f21,
     b_fu1, b_fu2, ln_nt, cent, use_fp8) = flags
    return {
        0: False,
        1: (not b_f10) and (not b_f20) and (not ln_nt[1]),
        2: (not b_sa1) and (not ln_nt[2]),
        3: False,
        4: (not b_f11) and (not b_f21) and (not ln_nt[4]),
        5: False,
    }


def _build(flags):
    (b_cv, b_ct, b_sa1, b_ca0, b_ca1, b_f10, b_f11, b_f20, b_f21,
     b_fu1, b_fu2, ln_nt, cent, use_fp8) = flags

    nc = bacc.Bacc("TRN2", target_bir_lowering=False, debug=False)

    # input pre-permuted host-side to [p][half][k][r] so each half loads as
    # one fully-contiguous-per-partition DMA (8 KiB lines, few descriptors)
    din0 = nc.dram_tensor("in0T", [P, 2 * KD * BLOC], BF16, kind="ExternalInput")
    cones = nc.dram_tensor("cones", [P, 2], F32R, kind="ExternalInput")
    cone1 = nc.dram_tensor("cone1", [1, P], F32R, kind="ExternalInput")
    wcv = nc.dram_tensor("wcv", [P, KD * KD * P], BF16, kind="ExternalInput")
    wct = nc.dram_tensor("wct", [P, KD * KD * P], BF16, kind="ExternalInput")
    wsa1 = nc.dram_tensor("wsa1", [P, KD * KD * P], BF16, kind="ExternalInput")
    wca = [nc.dram_tensor(f"wca{i}", [P, KD * KD * P], BF16, kind="ExternalInput")
           for i in range(L)]
    wdt = FP8 if use_fp8 else BF16
    wf1 = [nc.dram_tensor(f"wf1_{i}", [P, KD * KF * P], wdt, kind="ExternalInput")
           for i in range(L)]
    wf2 = [nc.dram_tensor(f"wf2_{i}", [P, KF * KD * P], wdt, kind="ExternalInput")
           for i in range(L)]
    wfu1 = nc.dram_tensor("wfu1", [P, 2 * KD * KD * P], BF16, kind="ExternalInput")
    wfu2 = nc.dram_tensor("wfu2", [P, KD * KD * P], BF16, kind="ExternalInput")
    outT = nc.dram_tensor("outT", [D, BLOC], BF16, kind="ExternalOutput")

    def opt(name, shape, cond):
        return nc.dram_tensor(name, shape, F32, kind="ExternalInput") if cond else None

    dbcv = opt("bcv", [P, KD], b_cv)
    dbct = opt("bct", [P, KD], b_ct)
    dbsa1 = opt("bsa1", [P, KD], b_sa1)
    dbca = [opt("bca0", [P, KD], b_ca0), opt("bca1", [P, KD], b_ca1)]
    dbf1 = [opt("bf1_0", [P, KF], b_f10), opt("bf1_1", [P, KF], b_f11)]
    dbf2 = [opt("bf2_0", [P, KD], b_f20), opt("bf2_1", [P, KD], b_f21)]
    dbfu1 = opt("bfu1", [P, KD], b_fu1)
    dbfu2 = opt("bfu2", [P, KD], b_fu2)
    any_ln = any(ln_nt)
    dlnp = opt("lnp", [P, KD * 24], any_ln)

    AT = mybir.AluOpType
    AF = mybir.ActivationFunctionType
    skipvar = _skipvar_flags(flags)

    with tile.TileContext(nc) as tc:
        import contextlib
        ctx = contextlib.ExitStack()
        with ctx:
            const = ctx.enter_context(tc.tile_pool(name="const", bufs=1))
            xp = ctx.enter_context(tc.tile_pool(name="xp", bufs=2))
            h1p = ctx.enter_context(tc.tile_pool(name="h1p", bufs=1))
            wbp = ctx.enter_context(tc.tile_pool(name="wbp", bufs=8))
            sqp = ctx.enter_context(tc.tile_pool(name="sqp", bufs=2))
            stp = ctx.enter_context(tc.tile_pool(name="stp", bufs=1))
            bcp = ctx.enter_context(tc.tile_pool(name="bcp", bufs=2))
            outp = ctx.enter_context(tc.tile_pool(name="outp", bufs=2))
            psA = ctx.enter_context(tc.tile_pool(name="psA", bufs=6, space="PSUM"))
            psR = ctx.enter_context(tc.tile_pool(name="psR", bufs=1, space="PSUM"))

            ones = const.tile([P, 2], F32R)
            nc.sync.dma_start(ones[:], cones[:])
            # input lands in two DMAs: the t half (consumed first) alone, so
            # in0's first matmuls don't wait for the whole 2 MiB
            din0_r = din0.rearrange("p (h k r) -> p h k r", h=2, k=KD)
            xin = const.tile([P, 2, KD, BLOC], BF16, tag="xin")
            nc.sync.dma_start(xin[:, 1], din0_r[:, 1])
            eps_t = const.tile([1, 1], F32)
            nc.vector.memset(eps_t[:], EPS)
            ones128 = const.tile([1, P], F32R)
            nc.sync.dma_start(ones128[:], cone1[:])

            # warm the PE (p-state ramp) while the first DMAs land; fp32
            # memset operands so the ramp starts without waiting on any DMA
            wsrc = const.tile([P, 2], F32)
            nc.vector.memset(wsrc[:], 1.0)
            wps = psA.tile([1, 2], F32, tag="mm", name="warm")
            for _ in range(24):
                nc.tensor.matmul(wps[:], lhsT=wsrc[:, 0:1], rhs=wsrc[:, 0:2],
                                 start=True, stop=True)

            def load_bias(dram):
                if dram is None:
                    return None
                t = const.tile([P, dram.shape[1]], F32, tag=dram.name)
                nc.sync.dma_start(t[:], dram[:])
                return t

            tbcv = load_bias(dbcv)
            tbct = load_bias(dbct)
            tbsa1 = load_bias(dbsa1)
            tbca = [load_bias(d) for d in dbca]
            tbf1 = [load_bias(d) for d in dbf1]
            tbf2 = [load_bias(d) for d in dbf2]
            tbfu1 = load_bias(dbfu1)
            tbfu2 = load_bias(dbfu2)
            tlnp = load_bias(dlnp)

            def new_gen(name):
                return [{h: xp.tile([P, BLOC], BF16, tag=f"x{k}_{h}",
                                    name=f"{name}{k}_{h}")
                         for h in range(2)} for k in range(KD)]

            # ---------------- LayerNorm machinery ----------------
            def ln_begin(step, n):
                need_mu = not cent[step]
                need_q = not skipvar[step]
                if not (need_mu or need_q):
                    return None
                if need_mu and need_q:
                    ps = psR.tile([33, 512], F32, tag=f"q{n}", name=f"q{step}{n}")
                    mu_ap, q_ap = ps[0:1, :], ps[32:33, :]
                elif need_mu:
                    ps = psR.tile([1, 512], F32, tag=f"q{n}", name=f"q{step}{n}")
                    mu_ap, q_ap = ps[0:1, :], None
                else:
                    ps = psR.tile([1, 512], F32, tag=f"q{n}", name=f"q{step}{n}")
                    mu_ap, q_ap = None, ps[0:1, :]
                return {"step": step, "n": n, "mu": mu_ap, "q": q_ap}

            def ln_chunk(st, Y, m, n, sq_on_dve=False):
                if st is None:
                    return
                yk = Y[m][n]
                step = st["step"]
                if st["mu"] is not None:
                    yc = sqp.tile([P, 512], F32R, tag="sq", name=f"yc{step}{n}{m}")
                    nc.scalar.activation(yc[:], yk[:], AF.Copy)
                    nc.tensor.matmul(st["mu"], lhsT=ones[:, 0:1], rhs=yc[:],
                                     start=(m == 0), stop=(m == KD - 1))
                if st["q"] is not None:
                    sq = sqp.tile([P, 512], F32R, tag="sq", name=f"sq{step}{n}{m}")
                    if sq_on_dve:
                        nc.vector.tensor_tensor(sq[:], yk[:], yk[:], op=AT.mult)
                    else:
                        nc.scalar.activation(sq[:], yk[:], AF.Square)
                    nc.tensor.matmul(st["q"], lhsT=ones[:, 1:2], rhs=sq[:],
                                     start=(m == 0), stop=(m == KD - 1))

            def ln_stats_apply(st, Y):
                if st is None:
                    return
                step, n = st["step"], st["n"]
                rb = cb = None
                if st["q"] is not None and st["mu"] is None:
                    sd = stp.tile([1, 512], F32, tag=f"sd{n}", name=f"sd{step}{n}")
                    nc.scalar.activation(sd[:], st["q"], AF.Sqrt,
                                         bias=eps_t[:], scale=1.0)
                    rstd = stp.tile([1, 512], F32, tag=f"rs{n}", name=f"rs{step}{n}")
                    nc.vector.reciprocal_approx_fast(rstd[:], sd[:])
                    rsr = stp.tile([1, 512], F32R, tag=f"rr{n}", name=f"rr{step}{n}")
                    nc.vector.tensor_copy(rsr[:], rstd[:])
                    # broadcast across partitions on the PE (ones ⊗ rstd):
                    # ~0.2us vs ~1.1us on gpsimd
                    rb = psA.tile([P, 512], F32, tag="mm", name=f"rb{step}{n}")
                    nc.tensor.matmul(rb[:], lhsT=ones128[:], rhs=rsr[:],
                                     start=True, stop=True)
                elif st["mu"] is not None and st["q"] is not None:
                    nm = stp.tile([1, 512], F32, tag=f"nm{n}", name=f"nm{step}{n}")
                    nc.scalar.activation(nm[:], st["mu"], AF.Copy)
                    t1 = stp.tile([1, 512], F32, tag=f"t1{n}", name=f"t1{step}{n}")
                    var = stp.tile([1, 512], F32, tag=f"va{n}", name=f"va{step}{n}")
                    rstd = stp.tile([1, 512], F32, tag=f"rs{n}", name=f"rs{step}{n}")
                    cc = stp.tile([1, 512], F32, tag=f"cc{n}", name=f"cc{step}{n}")
                    sd = stp.tile([1, 512], F32, tag=f"sd{n}", name=f"sd{step}{n}")
                    nc.vector.tensor_tensor(t1[:], nm[:], nm[:], op=AT.mult)
                    nc.vector.tensor_tensor(var[:], st["q"], t1[:], op=AT.subtract)
                    nc.scalar.activation(sd[:], var[:], AF.Sqrt,
                                         bias=eps_t[:], scale=1.0)
                    nc.vector.reciprocal_approx_fast(rstd[:], sd[:])
                    nc.vector.tensor_tensor(cc[:], nm[:], rstd[:], op=AT.mult)
                    rb = bcp.tile([P, 512], F32, tag=f"rb{n}", name=f"rb{step}{n}")
                    cb = bcp.tile([P, 512], F32, tag=f"cb{n}", name=f"cb{step}{n}")
                    nc.gpsimd.partition_broadcast(rb[:], rstd[:])
                    nc.gpsimd.partition_broadcast(cb[:], cc[:])
                else:  # mu only
                    nm = stp.tile([1, 512], F32, tag=f"nm{n}", name=f"nm{step}{n}")
                    nc.scalar.activation(nm[:], st["mu"], AF.Copy)
                    cb = bcp.tile([P, 512], F32, tag=f"cb{n}", name=f"cb{step}{n}")
                    nc.gpsimd.partition_broadcast(cb[:], nm[:])
                rb_in_psum = st["q"] is not None and st["mu"] is None
                for k in range(KD):
                    yk = Y[k][n]
                    eng = nc.vector if (rb_in_psum or k < 5) else nc.gpsimd
                    if rb is not None:
                        eng.tensor_tensor(yk[:], yk[:], rb[:], op=AT.mult)
                    if cb is not None:
                        eng.tensor_tensor(yk[:], yk[:], cb[:], op=AT.add)
                    if ln_nt[step]:
                        base = step * 4 * KD + (0 if n == 0 else 2 * KD)
                        g = tlnp[:, base + k:base + k + 1]
                        bb = tlnp[:, base + KD + k:base + KD + k + 1]
                        nc.vector.tensor_scalar(yk[:], in0=yk[:],
                                                scalar1=g, scalar2=bb,
                                                op0=AT.mult, op1=AT.add)

            # ---------------- op bodies ----------------
            def evict(kind, Yo, m, on, ps, bt, X=None):
                bias = bt[:, m:m + 1] if bt is not None else 0.0
                if kind == "res":
                    nc.vector.scalar_tensor_tensor(
                        Yo[m][on][:], in0=ps[:], scalar=bias,
                        in1=X[m][on][:], op0=AT.add, op1=AT.add)
                elif bt is not None:
                    nc.vector.tensor_scalar_add(Yo[m][on][:], in0=ps[:],
                                                scalar1=bias)
                else:
                    nc.scalar.activation(Yo[m][on][:], ps[:], AF.Copy)

            def linear_dd(X, wimg, bt, kind, Ynew, name, ln_step,
                          swap=False, nlist=(0, 1), xq=None):
                """m-outer [DxD] matmul: one weight block load serves both
                512-row slices.  kind 'copy' (psum->Y) or 'res' (+residual);
                swap crosses the modality halves.  xq: also emit fp8 copies
                of the output chunks (valid only when the step has no LN
                apply work, i.e. st is None)."""
                sts = {nn: ln_begin(ln_step, nn) for nn in (0, 1)} \
                    if ln_step is not None else {0: None, 1: None}
                for m in range(KD):
                    wt = wbp.tile([P, KD * P], BF16, tag="w", name=f"w{name}{m}")
                    nc.sync.dma_start(wt[:], wimg[:, m * KD * P:(m + 1) * KD * P])
                    for on in nlist:
                        rh = (1 - on) if swap else on
                        ps = psA.tile([P, 512], F32, tag="mm", name=f"p{name}{m}{on}")
                        for k in range(KD):
                            nc.tensor.matmul(
                                ps[:], lhsT=wt[:, k * P:(k + 1) * P],
                                rhs=X[k][rh][:], start=(k == 0),
                                stop=(k == KD - 1))
                        evict(kind, Ynew, m, on, ps, bt, X)
                        if xq is not None and sts[on] is None:
                            nc.scalar.activation(xq[on][:, m, :], Ynew[m][on][:],
                                                 AF.Copy)
                        ln_chunk(sts[on], Ynew, m, on, sq_on_dve=(kind == "copy"))
                for on in nlist:
                    ln_stats_apply(sts[on], Ynew)
                    if xq is not None and sts[on] is not None:
                        for m in range(KD):
                            nc.scalar.activation(xq[on][:, m, :], Ynew[m][on][:],
                                                 AF.Copy)

            def ffn_bf16(X, li, ln_step, nlist=(0, 1)):
                Ynew = new_gen(f"yf{li}")
                sts = {nn: ln_begin(ln_step, nn) for nn in (0, 1)} \
                    if ln_step is not None else {0: None, 1: None}
                h1 = [{} for _ in range(KF)]
                for m in range(KF):
                    wt = wbp.tile([P, KD * P], BF16, tag="w", name=f"wf1_{li}{m}")
                    nc.sync.dma_start(wt[:], wf1[li][:, m * KD * P:(m + 1) * KD * P])
                    for n in nlist:
                        ps = psA.tile([P, 512], F32, tag="mm", name=f"pf1_{li}{m}{n}")
                        for k in range(KD):
                            nc.tensor.matmul(
                                ps[:], lhsT=wt[:, k * P:(k + 1) * P],
                                rhs=X[k][n][:], start=(k == 0), stop=(k == KD - 1))
                        ht = h1p.tile([P, 512], BF16, tag=f"h{m}_{n}",
                                      name=f"h{li}{m}{n}")
                        bias = (tbf1[li][:, m:m + 1]
                                if tbf1[li] is not None else 0.0)
                        nc.scalar.activation(ht[:], ps[:], AF.Relu, bias=bias)
                        h1[m][n] = ht
                for m in range(KD):
                    wta = wbp.tile([P, 16 * P], BF16, tag="w", name=f"wf2a{li}{m}")
                    wtb = wbp.tile([P, 16 * P], BF16, tag="w", name=f"wf2b{li}{m}")
                    off = m * KF * P
                    nc.sync.dma_start(wta[:], wf2[li][:, off:off + 16 * P])
                    nc.sync.dma_start(wtb[:], wf2[li][:, off + 16 * P:off + 32 * P])
                    for n in nlist:
                        ps = psA.tile([P, 512], F32, tag="mm", name=f"pf2_{li}{m}{n}")
                        for k in range(KF):
                            wtk = wta if k < 16 else wtb
                            nc.tensor.matmul(
                                ps[:], lhsT=wtk[:, (k % 16) * P:(k % 16 + 1) * P],
                                rhs=h1[k][n][:], start=(k == 0), stop=(k == KF - 1))
                        bias = tbf2[li][:, m:m + 1] if tbf2[li] is not None else 0.0
                        nc.vector.scalar_tensor_tensor(
                            Ynew[m][n][:], in0=ps[:], scalar=bias,
                            in1=X[m][n][:], op0=AT.add, op1=AT.add)
                        ln_chunk(sts[n], Ynew, m, n)
                for n in nlist:
                    ln_stats_apply(sts[n], Ynew)
                return Ynew

            def ffn_fp8(X, xq, li, ln_step, nlist=(0, 1)):
                """fp8 DoubleRow FFN: rhs comes from the fp8 copies xq;
                residual comes from the bf16 X.  Weights are pre-scaled by
                WS host-side; evictions descale by 1/WS."""
                DR = mybir.MatmulPerfMode.DoubleRow
                Ynew = new_gen(f"yf{li}")
                sts = {nn: ln_begin(ln_step, nn) for nn in (0, 1)} \
                    if ln_step is not None else {0: None, 1: None}
                w1r = wf1[li].rearrange("p (m k c) -> p m k c", m=KF, k=2 * NP1)
                w2r = wf2[li].rearrange("p (m k c) -> p m k c", m=KD, k=2 * NP2)
                hq = {n: h1p.tile([P, KF, 512], FP8, tag=f"hq{n}",
                                  name=f"hq{li}{n}") for n in (0, 1)}
                for m in range(KF):
                    wt = wbp.tile([P, 2 * NP1, P], FP8, tag="w", name=f"wf1_{li}{m}")
                    nc.sync.dma_start(wt[:], w1r[:, m, :, :])
                    for n in nlist:
                        ps = psA.tile([P, 512], F32, tag="mm", name=f"pf1_{li}{m}{n}")
                        for j in range(NP1):
                            nc.tensor.matmul(
                                ps[:], lhsT=wt[:, 2 * j:2 * j + 2, :],
                                rhs=xq[n][:, 2 * j:2 * j + 2, :],
                                start=(j == 0), stop=(j == NP1 - 1), perf_mode=DR)
                        bias = (tbf1[li][:, m:m + 1]
                                if tbf1[li] is not None else 0.0)
                        nc.scalar.activation(hq[n][:, m, :], ps[:], AF.Relu,
                                             bias=bias, scale=1.0 / WS)
                for m in range(KD):
                    wt = wbp.tile([P, 2 * NP2, P], FP8, tag="w", name=f"wf2_{li}{m}")
                    nc.sync.dma_start(wt[:], w2r[:, m, :, :])
                    for n in nlist:
                        ps = psA.tile([P, 512], F32, tag="mm", name=f"pf2_{li}{m}{n}")
                        for j in range(NP2):
                            nc.tensor.matmul(
                                ps[:], lhsT=wt[:, 2 * j:2 * j + 2, :],
                                rhs=hq[n][:, 2 * j:2 * j + 2, :],
                                start=(j == 0), stop=(j == NP2 - 1), perf_mode=DR)
                        nc.vector.scalar_tensor_tensor(
                            Ynew[m][n][:], in0=ps[:], scalar=1.0 / WS,
                            in1=X[m][n][:], op0=AT.mult, op1=AT.add)
                        if tbf2[li] is not None:
                            nc.vector.tensor_scalar_add(
                                Ynew[m][n][:], in0=Ynew[m][n][:],
                                scalar1=tbf2[li][:, m:m + 1])
                        ln_chunk(sts[n], Ynew, m, n)
                for n in nlist:
                    ln_stats_apply(sts[n], Ynew)
                return Ynew

            # ---------------- layer 0 fused input-proj + self-attn ----------
            # t half first so its LN finishes earliest (ca0 consumes t rows
            # as the rhs of the v-half output first).
            Y = new_gen("y0")
            for half, (wimg, bt) in ((1, (wct, tbct)), (0, (wcv, tbcv))):
                if half == 0:
                    nc.sync.dma_start(xin[:, 0], din0_r[:, 0])
                st = ln_begin(0, half)
                for m in range(KD):
                    wt = wbp.tile([P, KD * P], BF16, tag="w",
                                  name=f"w0_{half}_{m}")
                    nc.sync.dma_start(
                        wt[:], wimg[:, m * KD * P:(m + 1) * KD * P])
                    ps = psA.tile([P, BLOC], F32, tag="mm",
                                  name=f"p0_{half}_{m}")
                    for k in range(KD):
                        nc.tensor.matmul(
                            ps[:], lhsT=wt[:, k * P:(k + 1) * P],
                            rhs=xin[:, half, k, :],
                            start=(k == 0), stop=(k == KD - 1))
                    evict("copy", Y, m, half, ps, bt)
                    ln_chunk(st, Y, m, half, sq_on_dve=True)
                ln_stats_apply(st, Y)

            # ---------------- layers (unrolled) ----------
            X = Y
            if use_fp8:
                xq0 = {n: h1p.tile([P, KD, 512], FP8, tag=f"xq{n}", bufs=2,
                                   name=f"xq0_{n}") for n in (0, 1)}
            else:
                xq0 = None
            Yc = new_gen("yc0")
            linear_dd(X, wca[0], tbca[0], "res", Yc, "ca0", 1,
                      swap=True, nlist=(0, 1), xq=xq0)
            if use_fp8:
                X = ffn_fp8(Yc, xq0, 0, 2, nlist=(0, 1))
            else:
                X = ffn_bf16(Yc, 0, 2, nlist=(0, 1))

            Ys = new_gen("ys1")
            linear_dd(X, wsa1, tbsa1, "copy", Ys, "sa1", 3, nlist=(1, 0))
            if use_fp8:
                xq1 = {n: h1p.tile([P, KD, 512], FP8, tag=f"xq{n}", bufs=2,
                                   name=f"xq1_{n}") for n in (0, 1)}
            else:
                xq1 = None
            Yc = new_gen("yc1")
            linear_dd(Ys, wca[1], tbca[1], "res", Yc, "ca1", 4,
                      swap=True, nlist=(0, 1), xq=xq1)
            if use_fp8:
                X = ffn_fp8(Yc, xq1, 1, 5, nlist=(1, 0))
            else:
                X = ffn_bf16(Yc, 1, 5, nlist=(1, 0))

            # ---------------- fusion head ----------
            # contraction order: t chunks first (their LN finished first)
            korder = list(range(KD, 2 * KD)) + list(range(KD))
            hf = []
            for mb in range(8):
                wt = wbp.tile([P, 2 * KD * P], BF16, tag="w", name=f"wfu1_{mb}")
                nc.sync.dma_start(
                    wt[:], wfu1[:, mb * 2 * KD * P:(mb + 1) * 2 * KD * P])
                ps = psA.tile([P, 512], F32, tag="mm", name=f"pfu1_{mb}")
                for j, k in enumerate(korder):
                    rhs = X[k][0][:] if k < KD else X[k - KD][1][:]
                    nc.tensor.matmul(
                        ps[:], lhsT=wt[:, k * P:(k + 1) * P],
                        rhs=rhs, start=(j == 0), stop=(j == 2 * KD - 1))
                ht = h1p.tile([P, 512], BF16, tag=f"hf{mb}", name=f"hf{mb}")
                bias = tbfu1[:, mb:mb + 1] if tbfu1 is not None else 0.0
                nc.scalar.activation(ht[:], ps[:], AF.Relu, bias=bias)
                hf.append(ht)
            for mb in range(4):
                wt = wbp.tile([P, 2 * KD * P], BF16, tag="w", name=f"wfu2_{mb}")
                nc.sync.dma_start(
                    wt[:], wfu2[:, mb * 2 * KD * P:(mb + 1) * 2 * KD * P])
                for mi in range(2):
                    m = mb * 2 + mi
                    ps = psA.tile([P, 512], F32, tag="mm", name=f"pfu2_{m}")
                    for k in range(KD):
                        nc.tensor.matmul(
                            ps[:],
                            lhsT=wt[:, (mi * KD + k) * P:(mi * KD + k + 1) * P],
                            rhs=hf[k][:], start=(k == 0), stop=(k == KD - 1))
                    ot = outp.tile([P, 512], BF16, tag="o", name=f"o{m}")
                    if tbfu2 is not None:
                        nc.vector.tensor_scalar_add(ot[:], in0=ps[:],
                                                    scalar1=tbfu2[:, m:m + 1])
                    else:
                        nc.vector.tensor_copy(ot[:], ps[:])
                    nc.sync.dma_start(outT[m * P:(m + 1) * P, :], ot[:])

    nc.compile()
    return nc


def _prep(inputs):
    """Host-side weight fusion + centering + lhsT image construction."""
    g = {k: np.asarray(v, dtype=np.float64) for k, v in inputs.items()}
    I = np.eye(D)
    C = I - 1.0 / D  # feature-centering projector

    def att_fuse(wqkv, bqkv, wo, bo):
        wv = wqkv[2 * D:]
        bv = bqkv[2 * D:]
        return wo @ wv, wo @ bv + bo

    Wsa, bsa, Wca, bca = [], [], [], []
    for i in range(L):
        w, b = att_fuse(g["sa_wqkv"][i], g["sa_bqkv"][i], g["sa_wo"][i], g["sa_bo"][i])
        Wsa.append(w); bsa.append(b)
        w, b = att_fuse(g["ca_wqkv"][i], g["ca_bqkv"][i], g["ca_wo"][i], g["ca_bo"][i])
        Wca.append(w); bca.append(b)

    # LN params per step; v-half params then t-half params.
    ln_steps = []
    for i in range(L):
        ln_steps.append((g["ln1g"][i], g["ln1b"][i], g["ln1g"][i], g["ln1b"][i]))
        ln_steps.append((g["ln2g"][i], g["ln2b"][i], g["ln3g"][i], g["ln3b"][i]))
        ln_steps.append((g["ln2g"][i], g["ln2b"][i], g["ln3g"][i], g["ln3b"][i]))
    ln_nt = tuple(
        not (np.all(gv == 1) and np.all(bv == 0) and np.all(gt == 1) and np.all(bt == 0))
        for (gv, bv, gt, bt) in ln_steps
    )
    # cent[s]: every activation entering step s's pre-LN sum is centered,
    # so centering the op's weights makes the feature-mean exactly zero.
    cent, ok = [], True
    for s in range(6):
        cent.append(ok)
        ok = ok and not ln_nt[s]
    cent = tuple(cent)

    M0 = I + Wsa[0]
    Wcv, Wct = M0 @ g["vw"], M0 @ g["tw"]
    bcv = M0 @ g["vb"] + bsa[0]
    bct = M0 @ g["tb"] + bsa[0]
    Wsa1 = I + Wsa[1]
    Wca0, bca0 = Wca[0], bca[0]
    Wca1, bca1 = Wca[1], bca[1]
    Wf2 = [g["fw2"][0], g["fw2"][1]]
    bf2 = [g["fb2"][0], g["fb2"][1]]
    bsa1 = bsa[1]
    if cent[0]:
        Wcv, Wct, bcv, bct = C @ Wcv, C @ Wct, C @ bcv, C @ bct
    if cent[1]:
        Wca0, bca0 = C @ Wca0, C @ bca0
    if cent[2]:
        Wf2[0], bf2[0] = C @ Wf2[0], C @ bf2[0]
    if cent[3]:
        Wsa1, bsa1 = C @ Wsa1, C @ bsa1
    if cent[4]:
        Wca1, bca1 = C @ Wca1, C @ bca1
    if cent[5]:
        Wf2[1], bf2[1] = C @ Wf2[1], C @ bf2[1]

    if FP8_FFN:
        f1img = [_img_lhsT_dr(g["fw1"][0]), _img_lhsT_dr(g["fw1"][1])]
        f2img = [_img_lhsT_dr(Wf2[0]), _img_lhsT_dr(Wf2[1])]
    else:
        f1img = [_img_lhsT(g["fw1"][0]), _img_lhsT(g["fw1"][1])]
        f2img = [_img_lhsT(Wf2[0]), _img_lhsT(Wf2[1])]

    weights = {
        "cones": np.stack([np.full(P, -1.0 / D), np.full(P, 1.0 / D)],
                          axis=1).astype(np.float32),
        "cone1": np.ones((1, P), np.float32),
        "wcv": _img_lhsT(Wcv), "wct": _img_lhsT(Wct), "wsa1": _img_lhsT(Wsa1),
        "wca0": _img_lhsT(Wca0), "wca1": _img_lhsT(Wca1),
        "wf1_0": f1img[0], "wf1_1": f1img[1],
        "wf2_0": f2img[0], "wf2_1": f2img[1],
        "wfu1": _img_lhsT(g["fus_w1"]), "wfu2": _img_lhsT(g["fus_w2"]),
    }

    def nz(x):
        return bool(np.any(x != 0.0))

    biases = {
        "bcv": bcv, "bct": bct, "bsa1": bsa1, "bca0": bca0, "bca1": bca1,
        "bf1_0": g["fb1"][0], "bf1_1": g["fb1"][1],
        "bf2_0": bf2[0], "bf2_1": bf2[1],
        "bfu1": g["fus_b1"], "bfu2": g["fus_b2"],
    }
    bflags = []
    for name in ("bcv", "bct", "bsa1", "bca0", "bca1", "bf1_0", "bf1_1",
                 "bf2_0", "bf2_1", "bfu1", "bfu2"):
        has = nz(biases[name])
        bflags.append(has)
        if has:
            weights[name] = _bcol(biases[name])

    if any(ln_nt):
        cols = []
        for (gv, bv, gt, bt) in ln_steps:
            cols += [_bcol(gv), _bcol(bv), _bcol(gt), _bcol(bt)]
        weights["lnp"] = np.concatenate(cols, axis=1)

    flags = tuple(bflags) + (ln_nt, cent, FP8_FFN)
    return weights, flags


def kernel(**inputs):
    vision = np.asarray(inputs["vision_features"], np.float32)
    text = np.asarray(inputs["text_features"], np.float32)

    weights, flags = _prep(inputs)
    if flags not in _cache:
        _cache[flags] = _build(flags)
    nc = _cache[flags]

    vis16 = vision.astype(NPBF16)
    txt16 = text.astype(NPBF16)
    in_maps = []
    for c in range(NCORES):
        rs = slice(c * BLOC, (c + 1) * BLOC)
        # [p][half][k][r] tile-image layout: contiguous per partition
        in0 = np.empty((P, 2, KD, BLOC), dtype=NPBF16)
        for half, src in ((0, vis16[rs]), (1, txt16[rs])):
            in0[:, half] = np.ascontiguousarray(src.T).reshape(
                KD, P, BLOC).transpose(1, 0, 2)
        m = dict(weights)
        m["in0T"] = in0.reshape(P, 2 * KD * BLOC)
        in_maps.append(m)

    res = run_bass_kernel_spmd(nc, in_maps, core_ids=list(range(NCORES)),
                               trace=TRACE, **TRACE_KW)
    kernel.last_result = res

    out = np.empty((B, D), dtype=np.float32)
    for c in range(NCORES):
        out[c * BLOC:(c + 1) * BLOC, :] = \
            res.results[c]["outT"].astype(np.float32).T
    return out
